# revision 12
# baseline (speedup 1.0000x reference)
"""Multi-head attention (AnyAttention) on 8 TRN2 NeuronCores.

Sharding: (batch, head-group): core i handles batch i//4 and heads
4*(i%4) .. 4*(i%4)+4 over ALL 2048 queries (tensor parallel on heads,
row-parallel output projection).  Each core emits a bf16 partial
output [2048, 1024] = attn_out_mine @ Wp[mine_rows]; the host sums
the 4 partials per batch in fp32.  No redundant K/V projection
compute (the previous kernel recomputed K/V 4x per batch): per-core
PE work drops from ~218us to ~155us, overlapping the ~142us of ACT
exp which is the other hard floor.

Per-core layout/tricks (measured 268us vs 382us for the old kernel):
  - qT/kT computed c-major [128(=2 heads x 64), 2048] by streaming x^T
    as matmul rhs (full-rate N=512 streams)
  - v computed token-major [128 tok, 4 heads, 65] (65th col = ones for
    the softmax denominator, folded into the PV matmul stream)
  - logits transposed S^T[k, q]: per (head-pair, qch=512, kt): both
    heads' QK matmuls at PE row bases 0/64 run concurrently into the
    two banks of one [128, 1024] psum tile; ONE exp instruction
    [128, 1024] covers both heads (the ~350-cycle ACT op overhead is
    the reason for the 1024-wide tiles); the per-pk psum j-order
    alternates so each QK reuses the buffer freed by the EARLY exp of
    the previous pair
  - PV trails QK/exp by 2 kt-pairs (LAG) so its mask-multiplied inputs
    are always ready and never head-of-line-block the strict PE FIFO;
    the 2 leftover pairs of each block ride in the next block's first
    slot (cross-block software pipeline)
  - mask applied post-exp as mul by host-prepped (1-mask)^T; et tiles
    laid out [128, head, ktslot, 512] so each mask mul is a single
    [128, 1024] bf16 2x-mode DVE op per head per kt-pair; the head-1
    muls of early kt-pairs run on the otherwise-idle GpSimd engine
    (its ~2.3us op latency fits the lag-2 deadline there)
  - softmax 1/denom via one batched nc.vector.reciprocal_approx_fast
    (staged to SBUF first -- the custom DVE op misreads PSUM), then a
    ones-row broadcast matmul; the final normalize multiply reads PV
    straight out of PSUM (WAR-gated against the next block's PV)
  - the bc matmul + normalize are deferred into the next block's
    interleave so the PE FIFO never waits on the DVE recip chain at a
    block boundary (HAM re-throttles the PE to 1.2 GHz after ~3.4us
    of idle, which doubles the cost of everything that follows)
  - projection / output-projection groups are interleaved (by emission
    order) into the ACT-bound attention slots via two utility PSUM
    banks; PSUM: QK [128,1024]x2 (4 banks) + PV [65,512]x2 (2) +
    utility [128,512]x2 (2)
  - DMA: one descriptor per tensor where possible (the Sync engine
    issues descriptors serially at ~650ns each, which gates startup);
    wq/wk/xt on the Sync queue, mask/wv/wp on the Scalar engine's
    hardware DGE queue; NEVER bulk data through the gpsimd software
    DGE (measured ~3x slower)
  - all matmuls bf16 with fp32 PSUM accumulation; scale 1/sqrt(c)
    folded into Wq on host; bp added on host
"""

import numpy as np
import ml_dtypes

B, N, D = 2, 2048, 1024
G, C = 16, 64          # heads, head dim
HPC = 4                # heads per core
NCORES = 8
NQCH = 4               # query chunks of 512
QCH = N // NQCH
TT = N // 128          # 16 token/key tiles
KT = D // 128          # 8 contraction tiles over d

BF16 = ml_dtypes.bfloat16

_cache = {}


def _import_concourse():
    try:
        import concourse.bass  # noqa: F401
    except ImportError:
        import sys
        sys.path.insert(0, "/opt/trn_rl_repo")


def _build():
    _import_concourse()
    import concourse.bass as bass  # noqa: F401
    from concourse import bacc, mybir
    import concourse.tile as tile

    fp32 = mybir.dt.float32
    bf16 = mybir.dt.bfloat16
    AF = mybir.ActivationFunctionType

    nc = bacc.Bacc("TRN2", target_bir_lowering=False, debug=False,
                   num_devices=NCORES)

    # ---- DRAM I/O (per-core shards; same program on all cores) ----
    xt = nc.dram_tensor("xt", [D, N], bf16, kind="ExternalInput").ap()
    wq = nc.dram_tensor("wq", [D, 256], bf16, kind="ExternalInput").ap()
    wk = nc.dram_tensor("wk", [D, 256], bf16, kind="ExternalInput").ap()
    wv = nc.dram_tensor("wv", [D, 256], bf16, kind="ExternalInput").ap()
    wp = nc.dram_tensor("wp", [256, D], bf16, kind="ExternalInput").ap()
    maskt = nc.dram_tensor("maskt", [NQCH, TT, 128, QCH], bf16,
                           kind="ExternalInput").ap()
    out = nc.dram_tensor("out", [N, D], bf16, kind="ExternalOutput").ap()

    GPSIMD_EVERY = 2  # every 2nd kt-pair, head-1 mask mul goes to GpSimd

    with tile.TileContext(nc) as tc:
        with (
            tc.tile_pool(name="wts", bufs=3) as wpool,
            tc.tile_pool(name="wpp", bufs=1) as wppool,
            tc.tile_pool(name="xtp", bufs=1) as xtpool,
            tc.tile_pool(name="maskp", bufs=2) as maskpool,
            tc.tile_pool(name="stay", bufs=1) as stay,
            tc.tile_pool(name="etp", bufs=4) as etpool,
            tc.tile_pool(name="pvsbp", bufs=2) as pvsbp,
            tc.tile_pool(name="small", bufs=2) as small,
            tc.tile_pool(name="outp", bufs=2) as outpool,
            tc.tile_pool(name="psqk", bufs=2, space="PSUM") as psqk,
            tc.tile_pool(name="pspv", bufs=1, space="PSUM") as pspv,
            tc.tile_pool(name="psu", bufs=2, space="PSUM") as psu,
        ):
            # ---------- DMA (priority order; one descriptor per tensor
            # where possible -- the Sync engine issues descriptors
            # serially at ~650ns each, so descriptor count gates startup)
            w_t = {}
            for name, dr, eng in (("wq", wq, nc.sync), ("wk", wk, nc.sync)):
                wt = wpool.tile([128, KT * 256], bf16, tag=name, name=name)
                eng.dma_start(
                    out=wt.rearrange("p (dk c) -> p dk c", dk=KT),
                    in_=dr.rearrange("(dk p) c -> p dk c", p=128))
                w_t[name] = [wt[:, dk * 256:(dk + 1) * 256]
                             for dk in range(KT)]
            # descriptor issue is ~650ns each and serial per engine, so
            # spread the startup DMAs across three otherwise-idle engines
            xt_t = []
            for dk in range(KT):
                t = xtpool.tile([128, N], bf16, tag=f"xt{dk}")
                eng = nc.sync if dk % 2 == 0 else nc.scalar
                eng.dma_start(out=t, in_=xt[dk * 128:(dk + 1) * 128, :])
                xt_t.append(t)
            for name, dr in (("wv", wv),):
                wt = wpool.tile([128, KT * 256], bf16, tag=name, name=name)
                nc.sync.dma_start(
                    out=wt.rearrange("p (dk c) -> p dk c", dk=KT),
                    in_=dr.rearrange("(dk p) c -> p dk c", p=128))
                w_t[name] = [wt[:, dk * 256:(dk + 1) * 256]
                             for dk in range(KT)]
            # mask for qch0
            mask_t = {}

            def load_mask(qch, eng=None):
                mt = maskpool.tile([128, TT * QCH], bf16, tag="mask")
                (eng or nc.sync).dma_start(
                    out=mt.rearrange("p (kt q) -> p kt q", kt=TT),
                    in_=maskt[qch].rearrange("kt p q -> p kt q"))
                mask_t[qch] = mt

            load_mask(0, eng=nc.scalar)
            wpt = wppool.tile([128, 2 * D], bf16, tag="wp", name="wp")
            nc.sync.dma_start(
                out=wpt.rearrange("p (r c) -> p r c", r=2),
                in_=wp.rearrange("(r p) c -> p r c", p=128))
            w_t["wp"] = [wpt[:, r * D:(r + 1) * D] for r in range(2)]

            # ones rows (partition 0 and 64) for the denominator
            # broadcast matmuls -- matmul requires lhsT and rhs at the
            # same base partition
            ones_row = small.tile([C + 1, C], bf16, tag="ones")
            nc.vector.memset(ones_row, 1.0)
            # touch Exp now so ACT_TABLE_LOAD is off the critical path
            warm = small.tile([1, C], bf16, tag="warm")
            nc.scalar.activation(out=warm, in_=ones_row[0:1, :], func=AF.Exp)

            qT = [stay.tile([128, N], bf16, tag=f"qT{hp}", name=f"qT{hp}")
                  for hp in range(2)]
            kTt = [stay.tile([128, N], bf16, tag=f"kT{hp}", name=f"kT{hp}")
                   for hp in range(2)]
            aT = [stay.tile([128, N], bf16, tag=f"aT{hp}", name=f"aT{hp}")
                  for hp in range(2)]
            v_t = [stay.tile([128, HPC, C + 1], bf16, tag=f"v{tt}",
                             name=f"v{tt}")
                   for tt in range(TT)]

            # ---------- worker emitters ----------
            # the projection groups are emitted in two 4-matmul halves at
            # consecutive slots: a full 8-MM group (~1.7us) exceeds the
            # per-slot PE slack (~0.9us) and pushes the next QK past the
            # exp deadline, costing an ACT bubble per occurrence
            def _proj_half(wname, dst, hp, ch, part, box):
                if part == 0:
                    box.clear()
                    box.append(psu.tile([128, QCH], fp32, tag="util",
                                        name="projps"))
                ps = box[0]
                for dk in range(part * 4, part * 4 + 4):
                    nc.tensor.matmul(
                        ps, w_t[wname][dk][:, hp * 128:(hp + 1) * 128],
                        xt_t[dk][:, ch * QCH:(ch + 1) * QCH],
                        start=(dk == 0), stop=(dk == KT - 1))
                if part == 1:
                    nc.vector.tensor_copy(
                        out=dst[hp][:, ch * QCH:(ch + 1) * QCH], in_=ps)

            def qproj_group(hp, qch, part=None, box=[]):
                if part is None:
                    _proj_half("wq", qT, hp, qch, 0, box)
                    _proj_half("wq", qT, hp, qch, 1, box)
                else:
                    _proj_half("wq", qT, hp, qch, part, box)

            def kproj_group(hp, ch, part=None, box=[]):
                if part is None:
                    _proj_half("wk", kTt, hp, ch, 0, box)
                    _proj_half("wk", kTt, hp, ch, 1, box)
                else:
                    _proj_half("wk", kTt, hp, ch, part, box)

            def vproj_group(tt):
                ps = psu.tile([128, QCH], fp32, tag="util")
                for dk in range(KT):
                    nc.tensor.matmul(
                        ps[:, 0:256],
                        xt_t[dk][:, tt * 128:(tt + 1) * 128],
                        w_t["wv"][dk],
                        start=(dk == 0), stop=(dk == KT - 1))
                vt = v_t[tt]
                nc.vector.memset(vt[:, :, C:C + 1], 1.0)
                nc.vector.tensor_copy(
                    out=vt[:, :, 0:C],
                    in_=ps[:, 0:256].rearrange("p (h c) -> p h c", c=C))

            osb0_t = [stay.tile([128, D], bf16, tag=f"osb0{t}",
                                name=f"osb0{t}") for t in range(4)]

            def outproj_h0_half(tt2):
                # hp0 contribution of the LAST qch, computed while the
                # final attention block is still running
                r0 = (NQCH - 1) * QCH + tt2 * 128
                for dch in range(2):
                    ps = psu.tile([128, QCH], fp32, tag="util")
                    nc.tensor.matmul(
                        ps, aT[0][:, r0:r0 + 128],
                        w_t["wp"][0][:, dch * QCH:(dch + 1) * QCH],
                        start=True, stop=True)
                    nc.vector.tensor_copy(
                        out=osb0_t[tt2][:, dch * QCH:(dch + 1) * QCH],
                        in_=ps)

            def outproj_h1_add(tt2):
                r0 = (NQCH - 1) * QCH + tt2 * 128
                osb = outpool.tile([128, D], bf16, tag="osb")
                for dch in range(2):
                    ps = psu.tile([128, QCH], fp32, tag="util")
                    nc.tensor.matmul(
                        ps, aT[1][:, r0:r0 + 128],
                        w_t["wp"][1][:, dch * QCH:(dch + 1) * QCH],
                        start=True, stop=True)
                    nc.vector.tensor_add(
                        osb[:, dch * QCH:(dch + 1) * QCH], ps,
                        osb0_t[tt2][:, dch * QCH:(dch + 1) * QCH])
                nc.sync.dma_start(out=out[r0:r0 + 128, :], in_=osb)

            def outproj_group(qch, tt2):
                # one 128-token row block of the partial output
                osb = outpool.tile([128, D], bf16, tag="osb")
                r0 = qch * QCH + tt2 * 128
                for dch in range(2):
                    ps = psu.tile([128, QCH], fp32, tag="util")
                    for hp in range(2):
                        nc.tensor.matmul(
                            ps, aT[hp][:, r0:r0 + 128],
                            w_t["wp"][hp][:, dch * QCH:(dch + 1) * QCH],
                            start=(hp == 0), stop=(hp == 1))
                    nc.vector.tensor_copy(
                        out=osb[:, dch * QCH:(dch + 1) * QCH], in_=ps)
                nc.sync.dma_start(out=out[r0:r0 + 128, :], in_=osb)

            # ---------- attention block for (qch, hp): 16 kt slots ----------
            # The PV matmuls trail QK/exp by LAG kt-pairs so their
            # mask-multiplied inputs are always ready.  The LAG leftover
            # PV pairs at a block's end are NOT emitted here -- they are
            # returned as carry closures that the NEXT block weaves into
            # its first slots, so the ACT exp cadence never pauses at a
            # block boundary.
            LAG = 2

            def attn_block(qch, hp, interleave, carry_in):
                mt = mask_t[qch]
                pv = [pspv.tile([C + 1, QCH], fp32, tag=f"pv{h2}",
                                name=f"pv{h2}") for h2 in range(2)]
                et_tiles = [None] * (TT // 2)

                def pv_pair(pk2, pv=pv, hp=hp):
                    etp = et_tiles[pk2]
                    for j in range(2):
                        kt = 2 * pk2 + j
                        for h2 in range(2):
                            nc.tensor.matmul(
                                pv[h2], v_t[kt][:, hp * 2 + h2, :],
                                etp[:, h2, j, :],
                                start=(kt == 0), stop=(kt == TT - 1))

                for pk in range(TT // 2):
                    et = etpool.tile([128, 2, 2, QCH], bf16, tag="et")
                    et_tiles[pk] = et
                    for j in ((0, 1) if pk % 2 == 0 else (1, 0)):
                        kt = 2 * pk + j
                        ps = psqk.tile([128, 2 * QCH], fp32, tag="qk")
                        for h2 in range(2):
                            pb = h2 * C
                            nc.tensor.matmul(
                                ps[:, h2 * QCH:(h2 + 1) * QCH],
                                kTt[hp][pb:pb + C,
                                        kt * 128:(kt + 1) * 128],
                                qT[hp][pb:pb + C,
                                       qch * QCH:(qch + 1) * QCH],
                                start=True, stop=True)
                        nc.scalar.activation(
                            out=et[:, :, j, :],
                            in_=ps.rearrange("p (h q) -> p h q", h=2),
                            func=AF.Exp)
                    mslice = mt[:, 2 * pk * QCH:(2 * pk + 2) * QCH]
                    for h2 in range(2):
                        etf = et[:, h2].rearrange("p a b -> p (a b)")
                        # head-1 muls of the early kt-pairs go to the
                        # otherwise-idle GpSimd engine (its ~2.3us op
                        # latency fits the PV lag-2 deadline there; the
                        # late pairs feed the block tail and must stay on
                        # the faster DVE)
                        if h2 == 1 and pk <= TT // 2 - 3:
                            nc.gpsimd.tensor_mul(etf, etf, mslice)
                        else:
                            nc.vector.tensor_mul(etf, etf, mslice)
                    if pk == 0 and carry_in:
                        # previous block's leftover PV work + its pv-bank
                        # handoff ride in this block's first slot
                        for f in carry_in:
                            f()
                    if pk >= LAG:
                        pv_pair(pk - LAG)
                    for f in interleave.get(pk, ()):
                        f()

                def norm_part1(pv=pv):
                    # reciprocal of the two denominator rows; heads at
                    # partitions 0 and 64 (32-aligned) so the recip runs
                    # at FD=512 instead of FD=1024 on a single lane
                    dga = small.tile([C + 1, QCH], fp32, tag="dga")
                    for h2 in range(2):
                        nc.vector.tensor_copy(
                            out=dga[h2 * C:h2 * C + 1, :],
                            in_=pv[h2][C:C + 1, :])
                    rcf = small.tile([C + 1, QCH], fp32, tag="rcf")
                    nc.vector.reciprocal_approx_fast(rcf, dga)
                    rcb = small.tile([C + 1, QCH], bf16, tag="rcb")
                    nc.vector.tensor_copy(out=rcb, in_=rcf)
                    return rcb

                def make_finalize(rcb_box, pv=pv, hp=hp, qch=qch):
                    def finalize():
                        rcb = rcb_box[0]
                        bc = psu.tile([128, QCH], fp32, tag="util")
                        for h2 in range(2):
                            nc.tensor.matmul(
                                bc[h2 * C:(h2 + 1) * C, :],
                                ones_row[h2 * C:h2 * C + 1, :],
                                rcb[h2 * C:h2 * C + 1, :],
                                start=True, stop=True)
                        bcs = pvsbp.tile([128, QCH], bf16, tag="bcs",
                                         name="bcs")
                        nc.vector.tensor_copy(out=bcs, in_=bc)
                        # normalize straight out of the PV psum banks; the
                        # next block's first PV (start=True) is WAR-gated
                        # on these reads by the tile framework
                        for h2 in range(2):
                            nc.vector.tensor_mul(
                                aT[hp][h2 * C:(h2 + 1) * C,
                                       qch * QCH:(qch + 1) * QCH],
                                pv[h2][0:C, :],
                                bcs[h2 * C:(h2 + 1) * C, :])
                    return finalize

                rcb_box = []
                fin_holder = [make_finalize(rcb_box)]

                def tail0():
                    pv_pair(TT // 2 - LAG)

                def tail1():
                    pv_pair(TT // 2 - LAG + 1)
                    rcb_box.append(norm_part1())

                return [tail0, tail1], fin_holder

            # ---------- emission schedule ----------
            # prologue: just enough for (qch0, hp0) to start
            qproj_group(0, 0)
            kproj_group(0, 0)

            carry, fh_prev = None, None
            for qch in range(NQCH):
                if qch + 1 < NQCH:
                    load_mask(qch + 1)
                for hp in range(2):
                    il = {}

                    def add(pk, f):
                        il.setdefault(pk, []).append(f)

                    if fh_prev is not None:
                        # finalize of the previous block: its recip ran in
                        # this block's pk0 carry, so pk1 is the earliest
                        add(1, lambda fh=fh_prev: fh[0]())
                    if qch == 0 and hp == 0:
                        # k chunks c needed by QK at pk=2c; v tiles
                        # (2pk, 2pk+1) needed by PV at pk+2
                        add(0, lambda: vproj_group(0))
                        add(0, lambda: vproj_group(1))
                        add(0, lambda: kproj_group(0, 1))
                        for pk in range(1, 8):
                            add(pk, lambda t=2 * pk: vproj_group(t))
                            add(pk, lambda t=2 * pk + 1: vproj_group(t))
                        add(1, lambda: kproj_group(0, 2))
                        add(2, lambda: kproj_group(0, 3))
                        # hp1's first chunk + queries before block (0,1)
                        add(6, lambda: kproj_group(1, 0))
                        add(7, lambda: qproj_group(1, 0))
                    if qch == 0 and hp == 1:
                        b1, b2, b3 = [], [], []
                        add(0, lambda: kproj_group(1, 1, 0, b1))
                        add(1, lambda: kproj_group(1, 1, 1, b1))
                        add(2, lambda: kproj_group(1, 2, 0, b2))
                        add(3, lambda: kproj_group(1, 2, 1, b2))
                        add(4, lambda: kproj_group(1, 3, 0, b3))
                        add(5, lambda: kproj_group(1, 3, 1, b3))
                    if hp == 0 and qch > 0:
                        # output projection of the previous qch (both its
                        # finalizes have run by pk3)
                        for t in range(4):
                            add(t + 3,
                                lambda q=qch - 1, t=t: outproj_group(q, t))
                        bq = []
                        add(6, lambda q=qch: qproj_group(1, q, 0, bq))
                        add(7, lambda q=qch: qproj_group(1, q, 1, bq))
                    if hp == 1 and qch + 1 < NQCH:
                        bq0 = []
                        add(0, lambda q=qch: qproj_group(0, q + 1, 0, bq0))
                        add(1, lambda q=qch: qproj_group(0, q + 1, 1, bq0))
                    if hp == 1 and qch == NQCH - 1:
                        # last block: fin(3, hp0) ran at pk1, so the hp0
                        # halves of the last output projection fit here
                        for t in range(4):
                            add(t + 3, lambda t=t: outproj_h0_half(t))
                    carry, fh_prev = attn_block(qch, hp, il, None)
                    carry[0]()
                    carry[1]()
                    carry = None
            fh_prev[0]()
            for tt2 in range(4):
                outproj_h1_add(tt2)

    nc.compile()
    return nc


def _get_nc():
    if "nc" not in _cache:
        _cache["nc"] = _build()
    return _cache["nc"]


def _make_in_maps(x, mask, Wq, Wk, Wv, Wp):
    x = np.asarray(x, dtype=np.float32)
    mask = np.asarray(mask)
    scale = C ** (-0.5)
    wq_b = (np.asarray(Wq, np.float32) * scale).astype(BF16)
    wk_b = np.asarray(Wk, np.float32).astype(BF16)
    wv_b = np.asarray(Wv, np.float32).astype(BF16)
    wp_b = np.asarray(Wp, np.float32).astype(BF16)

    xTs, maskts = [], []
    for bi in range(B):
        xTs.append(np.ascontiguousarray(x[bi].T).astype(BF16))
        mt = (1 - mask[bi, :, 0, :]).T.astype(np.float32)  # [k, q]
        # -> [qch, kt, 128, 512]
        m4 = mt.reshape(TT, 128, NQCH, QCH).transpose(2, 0, 1, 3)
        maskts.append(np.ascontiguousarray(m4).astype(BF16))

    in_maps = []
    for core in range(NCORES):
        bi, hg = core // HPC, core % HPC
        cr = slice(256 * hg, 256 * (hg + 1))
        in_maps.append({
            "xt": xTs[bi],
            "wq": np.ascontiguousarray(wq_b[:, cr]),
            "wk": np.ascontiguousarray(wk_b[:, cr]),
            "wv": np.ascontiguousarray(wv_b[:, cr]),
            "wp": np.ascontiguousarray(wp_b[cr, :]),
            "maskt": maskts[bi],
        })
    return in_maps


def _run_once(nc, in_maps):
    _import_concourse()
    from concourse.bass_utils import run_bass_kernel_spmd

    res = run_bass_kernel_spmd(nc, in_maps, core_ids=list(range(NCORES)))
    full = np.zeros((B, N, D), np.float32)
    for core in range(NCORES):
        bi = core // HPC
        full[bi] += np.asarray(res.results[core]["out"], np.float32)
    return full


def kernel(x, mask, Wq, Wk, Wv, Wp, bp):
    nc = _get_nc()
    in_maps = _make_in_maps(x, mask, Wq, Wk, Wv, Wp)

    # The device very occasionally returns corrupted results right after a
    # runtime error; run twice and require agreement.
    a = _run_once(nc, in_maps)
    for _ in range(3):
        b = _run_once(nc, in_maps)
        da = np.linalg.norm(a - b) / max(1e-30, np.linalg.norm(b))
        if da < 1e-4:
            break
        a = b
    full = b
    full += np.asarray(bp, np.float32)[None, None, :]
    return full
